# revision 2
# baseline (speedup 1.0000x reference)
"""Port-Hamiltonian model forward pass (dstate/dt) as a Bass/Tile kernel on
8 TRN2 NeuronCores, pure data-parallel over the batch.

Math (per sample, feature-major / transposed layout on chip):
    z1T = W1.T sT + b1                  [512, n]
    h1T = softplus(z1T) = Ln(Exp(z1T)+1)
    z2T = W2.T h1T + b2
    s2T = sigmoid(z2T)                  (W3 folded into backward weights)
    uT  = (W2 * w3) s2T                 = dH/dh1 transposed
    g1T = uT * sigmoid(z1T)             = dH/dz1 transposed
    outT = (M @ W1) g1T + GM @ [a_hi; a_lo; 1]
  where M = [[0, 1], [-1, -damping]], GM carries Gw (for G_u) and Gb.

All big matmuls in bf16 (1 cyc/row on PE; fp32 is 4). z1 accuracy is
recovered with a hi/lo bf16 split of state and W1 packed into the K dim
(K=7: W1hi*xhi + W1hi*xlo + W1lo*xhi + b1*1).  z2T is stored in SBUF (bf16)
across the activation-table switch (exp/ln set -> sigmoid set); z1 is
recomputed in stage B instead of stored (4 matmuls/slice is cheaper than the
DVE copy + 64KB/partition of SBUF).
"""

import numpy as np
import ml_dtypes

B = 131072
S = 2
H = 512
E = 8
NCORES = 8
BC = B // NCORES  # 16384 samples per core
NSLICE = 512      # batch slice (matmul moving free dim / PSUM bank)
NS = BC // NSLICE  # 32 slices
HC = H // 128      # 4 hidden-dim chunks of 128 partitions

BF16 = ml_dtypes.bfloat16

_cached = {}
last_results = None  # test.py introspects this for profiling info


def _build_nc():
    import concourse.bacc as bacc
    import concourse.mybir as mybir
    import concourse.tile as tile

    f32 = mybir.dt.float32
    bf16 = mybir.dt.bfloat16
    ADD = mybir.AluOpType.add
    MULT = mybir.AluOpType.mult
    EXP = mybir.ActivationFunctionType.Exp
    LN = mybir.ActivationFunctionType.Ln
    SIG = mybir.ActivationFunctionType.Sigmoid

    nc = bacc.Bacc("TRN2", target_bir_lowering=False, debug=False)

    xT_d = nc.dram_tensor("xT", [7, BC], bf16, kind="ExternalInput")
    aT_d = nc.dram_tensor("aT", [17, BC], bf16, kind="ExternalInput")
    w1a_d = nc.dram_tensor("w1a", [7, H], bf16, kind="ExternalInput")
    w2_d = nc.dram_tensor("w2", [128, HC, H], bf16, kind="ExternalInput")
    w2wt_d = nc.dram_tensor("w2wt", [128, HC, H], bf16, kind="ExternalInput")
    w1ft_d = nc.dram_tensor("w1ft", [128, HC, S], bf16, kind="ExternalInput")
    gm_d = nc.dram_tensor("gm", [17, S], bf16, kind="ExternalInput")
    b2v_d = nc.dram_tensor("b2v", [128, HC], f32, kind="ExternalInput")
    outT_d = nc.dram_tensor("outT", [S, BC], f32, kind="ExternalOutput")

    with tile.TileContext(nc) as tc:
        with (
            tc.tile_pool(name="consts", bufs=1) as consts,
            tc.tile_pool(name="work", bufs=2) as work,
            tc.tile_pool(name="ps", bufs=8, space="PSUM") as ps,
        ):
            # ---- constants ----
            w1a = consts.tile([7, H], bf16)
            nc.sync.dma_start(w1a[:], w1a_d[:])
            w2 = consts.tile([128, HC, H], bf16)
            nc.sync.dma_start(w2[:], w2_d[:])
            w2wt = consts.tile([128, HC, H], bf16)
            nc.sync.dma_start(w2wt[:], w2wt_d[:])
            w1ft = consts.tile([128, HC, S], bf16)
            nc.sync.dma_start(w1ft[:], w1ft_d[:])
            gm = consts.tile([17, S], bf16)
            nc.sync.dma_start(gm[:], gm_d[:])
            b2v = consts.tile([128, HC], f32)
            nc.sync.dma_start(b2v[:], b2v_d[:])

            # z2 (bf16) for the whole core batch, persisted across the
            # activation-table switch: [partition, hidden-chunk, slice, col]
            z2s = consts.tile([128, HC, NS, NSLICE], bf16)

            def z1_matmuls(x_t, tag):
                """4 accumulation-free matmuls -> list of [128, NSLICE] psum
                chunks holding z1T (+b1) for this slice."""
                chunks = []
                for jc in range(HC):
                    zp = ps.tile([128, NSLICE], f32, tag="ps", name=f"{tag}_{jc}")
                    nc.tensor.matmul(
                        zp[:],
                        w1a[:, jc * 128 : (jc + 1) * 128],
                        x_t[:],
                        start=True,
                        stop=True,
                    )
                    chunks.append(zp)
                return chunks

            # ================= stage A: forward to z2 =================
            for n in range(NS):
                csl = slice(n * NSLICE, (n + 1) * NSLICE)
                x_t = work.tile([7, NSLICE], bf16, tag="xa", bufs=3, name=f"xa{n}")
                nc.sync.dma_start(x_t[:], xT_d[:, csl])

                z1p = z1_matmuls(x_t, f"z1p{n}")

                # e1 = Exp(z1), chunks into one tile; h1 = Ln(e1 + 1) = softplus
                e1 = work.tile([128, HC, NSLICE], f32, tag="e1", bufs=1, name=f"e1_{n}")
                for jc in range(HC):
                    nc.scalar.activation(e1[:, jc, :], z1p[jc][:], EXP)
                h1 = work.tile([128, HC, NSLICE], bf16, tag="h1", bufs=2, name=f"h1_{n}")
                nc.scalar.activation(h1[:], e1[:], LN, bias=1.0)

                # z2 chunk ic = sum_kc W2[kc,ic].T @ h1[kc]; store (+b2) as bf16
                for ic in range(HC):
                    z2p = ps.tile([128, NSLICE], f32, tag="ps", name=f"z2p{n}_{ic}")
                    for kc in range(HC):
                        nc.tensor.matmul(
                            z2p[:],
                            w2[:, kc, ic * 128 : (ic + 1) * 128],
                            h1[:, kc, :],
                            start=(kc == 0),
                            stop=(kc == HC - 1),
                        )
                    nc.vector.tensor_scalar(
                        z2s[:, ic, n, :], z2p[:], b2v[:, ic : ic + 1], None, ADD
                    )

            # ================= stage B: backward to output =================
            for n in range(NS):
                csl = slice(n * NSLICE, (n + 1) * NSLICE)
                x_t = work.tile([7, NSLICE], bf16, tag="xa", bufs=3, name=f"xb{n}")
                nc.sync.dma_start(x_t[:], xT_d[:, csl])
                a_t = work.tile([17, NSLICE], bf16, tag="aa", bufs=3, name=f"aa{n}")
                nc.sync.dma_start(a_t[:], aT_d[:, csl])

                z1q = z1_matmuls(x_t, f"z1q{n}")

                sg1 = work.tile(
                    [128, HC, NSLICE], bf16, tag="sg1", bufs=2, name=f"sg1_{n}"
                )
                for jc in range(HC):
                    nc.scalar.activation(sg1[:, jc, :], z1q[jc][:], SIG)

                sg2 = work.tile(
                    [128, HC, NSLICE], bf16, tag="sg2", bufs=2, name=f"sg2_{n}"
                )
                nc.scalar.activation(sg2[:], z2s[:, :, n, :], SIG)

                # u chunk ic = sum_jc (W2*w3).T[jc,ic].T @ sg2[jc];  g1 = u*sg1
                g1 = work.tile([128, HC, NSLICE], bf16, tag="g1", bufs=2, name=f"g1_{n}")
                for ic in range(HC):
                    up = ps.tile([128, NSLICE], f32, tag="ps", name=f"up{n}_{ic}")
                    for jc in range(HC):
                        nc.tensor.matmul(
                            up[:],
                            w2wt[:, jc, ic * 128 : (ic + 1) * 128],
                            sg2[:, jc, :],
                            start=(jc == 0),
                            stop=(jc == HC - 1),
                        )
                    nc.vector.tensor_mul(g1[:, ic, :], up[:], sg1[:, ic, :])

                # outT slice = (M@W1) @ g1 + GM.T @ a  (accumulated in psum)
                op = ps.tile([S, NSLICE], f32, tag="ps", name=f"op{n}")
                for kc in range(HC):
                    nc.tensor.matmul(
                        op[:],
                        w1ft[:, kc, :],
                        g1[:, kc, :],
                        start=(kc == 0),
                        stop=False,
                        skip_group_check=True,
                    )
                nc.tensor.matmul(
                    op[:], gm[:], a_t[:], start=False, stop=True, skip_group_check=True
                )
                o_t = work.tile([S, NSLICE], f32, tag="osb", bufs=3, name=f"ot{n}")
                nc.vector.tensor_copy(o_t[:], op[:])
                nc.sync.dma_start(outT_d[:, csl], o_t[:])

    nc.compile()
    return nc


def _hi_lo(a32):
    hi = a32.astype(BF16)
    lo = (a32 - hi.astype(np.float32)).astype(BF16)
    return hi, lo


def kernel(
    t,
    state,
    action_emb,
    W1,
    b1,
    W2,
    b2,
    W3,
    b3,
    log_damping,
    Gw,
    Gb,
):
    global last_results
    from concourse.bass_utils import run_bass_kernel_spmd

    state = np.asarray(state, dtype=np.float32)
    action_emb = np.asarray(action_emb, dtype=np.float32)
    W1 = np.asarray(W1, dtype=np.float32)
    b1 = np.asarray(b1, dtype=np.float32)
    W2 = np.asarray(W2, dtype=np.float32)
    b2 = np.asarray(b2, dtype=np.float32)
    W3 = np.asarray(W3, dtype=np.float32)
    b3 = np.asarray(b3, dtype=np.float32)  # unused: constant shift, no grad
    damping = float(np.exp(np.float32(log_damping)))
    Gw = np.asarray(Gw, dtype=np.float32)
    Gb = np.asarray(Gb, dtype=np.float32)

    # ---- host-side weight prep (tiny) ----
    w3col = W3[:, 0]
    w1hi, w1lo = _hi_lo(W1)  # [2, H] each
    w1a = np.concatenate(
        [w1hi, w1hi, w1lo, b1[None, :].astype(BF16)], axis=0
    )  # [7, H]

    w2r = (
        W2.astype(BF16).reshape(HC, 128, H).transpose(1, 0, 2).copy()
    )  # [128, HC, H]; [p, kc, i] = W2[kc*128+p, i]
    w2wt = (W2.T * w3col[:, None]).astype(BF16)  # [H(j), H(i)]
    w2wtr = w2wt.reshape(HC, 128, H).transpose(1, 0, 2).copy()

    M = np.array([[0.0, 1.0], [-1.0, -damping]], dtype=np.float32)
    w1f = M @ W1  # [2, H]
    w1ftr = w1f.T.astype(BF16).reshape(HC, 128, S).transpose(1, 0, 2).copy()

    gmat = np.zeros((17, S), dtype=np.float32)
    gmat[0:8, 1] = Gw[:, 0]
    gmat[8:16, 1] = Gw[:, 0]
    gmat[16, 1] = Gb[0]
    gmat = gmat.astype(BF16)

    b2v = np.ascontiguousarray(b2.reshape(HC, 128).T)  # [128, HC] f32

    # ---- per-core input shards ----
    sT = state.T  # [2, B]
    shi, slo = _hi_lo(sT)
    ones_row = np.ones((1, B), dtype=BF16)
    xT = np.concatenate([shi, slo, shi, ones_row], axis=0)  # [7, B]

    aT32 = action_emb.T  # [8, B]
    ahi, alo = _hi_lo(aT32)
    aT = np.concatenate([ahi, alo, ones_row], axis=0)  # [17, B]

    if "nc" not in _cached:
        _cached["nc"] = _build_nc()
    nc = _cached["nc"]

    in_maps = []
    for c in range(NCORES):
        csl = slice(c * BC, (c + 1) * BC)
        in_maps.append(
            {
                "xT": np.ascontiguousarray(xT[:, csl]),
                "aT": np.ascontiguousarray(aT[:, csl]),
                "w1a": w1a,
                "w2": w2r,
                "w2wt": w2wtr,
                "w1ft": w1ftr,
                "gm": gmat,
                "b2v": b2v,
            }
        )

    import os

    trace = bool(os.environ.get("PH_TRACE"))
    res = run_bass_kernel_spmd(
        nc, in_maps, core_ids=list(range(NCORES)), trace=trace
    )
    last_results = res

    out = np.empty((B, S), dtype=np.float32)
    for c in range(NCORES):
        out[c * BC : (c + 1) * BC, :] = res.results[c]["outT"].T
    return out


# revision 3
# speedup vs baseline: 1.3290x; 1.3290x over previous
"""Port-Hamiltonian model forward pass (dstate/dt) as a Bass/Tile kernel on
8 TRN2 NeuronCores, pure data-parallel over the batch.

Math (per sample, feature-major / transposed layout on chip):
    z1T = W1.T sT + b1                  [512, n]
    h1T = softplus(z1T) = Ln(Exp(z1T)+1)
    z2T = W2.T h1T + b2
    s2T = sigmoid(z2T)                  (W3 folded into backward weights)
    uT  = (W2 * w3) s2T                 = dH/dh1 transposed
    g1T = uT * sigmoid(z1T)             = dH/dz1 transposed
    outT = (M @ W1) g1T + GM @ [a_hi; a_lo; 1]
  where M = [[0, 1], [-1, -damping]], GM carries Gw (for G_u) and Gb.

All matmuls in bf16 (1 cyc/row on PE; fp32 is 4). z1 accuracy is recovered
with a hi/lo bf16 split of state and W1 packed into the K dim (K=7:
W1hi*xhi + W1hi*xlo + W1lo*xhi + b1*1), and the 4 hidden-chunk z1 matmuls
run CONCURRENTLY in the PE array via row tiling (tile_position=(32j,0),
each writing a different PSUM bank).  z2T is stored in SBUF (bf16) across
the activation-table switch (exp/ln set -> sigmoid set); z1 is recomputed
in stage B instead of stored (4 packed matmuls/slice beat the DVE copy +
64KB/partition of SBUF).
"""

import numpy as np
import ml_dtypes

B = 131072
S = 2
H = 512
E = 8
NCORES = 8
BC = B // NCORES  # 16384 samples per core
NSLICE = 512      # batch slice (matmul moving free dim / PSUM bank)
NS = BC // NSLICE  # 32 slices
HC = H // 128      # 4 hidden-dim chunks of 128 partitions

BF16 = ml_dtypes.bfloat16

_cached = {}
last_results = None  # test.py introspects this for profiling info


def _build_nc():
    import concourse.bacc as bacc
    import concourse.mybir as mybir
    import concourse.tile as tile

    f32 = mybir.dt.float32
    bf16 = mybir.dt.bfloat16
    ADD = mybir.AluOpType.add
    EXP = mybir.ActivationFunctionType.Exp
    LN = mybir.ActivationFunctionType.Ln
    SIG = mybir.ActivationFunctionType.Sigmoid

    nc = bacc.Bacc("TRN2", target_bir_lowering=False, debug=False)

    xT_d = nc.dram_tensor("xT", [7, BC], bf16, kind="ExternalInput")
    aT_d = nc.dram_tensor("aT", [17, BC], bf16, kind="ExternalInput")
    # W1-aug row-tiled: rows 32j+r (r<7) hold [W1hi;W1hi;W1lo;b1][r, 128j:128j+128]
    w1rt_d = nc.dram_tensor("w1rt", [128, 128], bf16, kind="ExternalInput")
    w2_d = nc.dram_tensor("w2", [128, HC, H], bf16, kind="ExternalInput")
    w2wt_d = nc.dram_tensor("w2wt", [128, HC, H], bf16, kind="ExternalInput")
    w1ft_d = nc.dram_tensor("w1ft", [128, HC, S], bf16, kind="ExternalInput")
    gm_d = nc.dram_tensor("gm", [17, S], bf16, kind="ExternalInput")
    b2v_d = nc.dram_tensor("b2v", [128, HC], f32, kind="ExternalInput")
    outT_d = nc.dram_tensor("outT", [S, BC], f32, kind="ExternalOutput")

    with tile.TileContext(nc) as tc:
        with (
            tc.tile_pool(name="consts", bufs=1) as consts,
            tc.tile_pool(name="work", bufs=2) as work,
            tc.tile_pool(name="ps", bufs=1, space="PSUM") as ps,
        ):
            # ---- constants ----
            w1rt = consts.tile([128, 128], bf16)
            nc.sync.dma_start(w1rt[:], w1rt_d[:])
            w2 = consts.tile([128, HC, H], bf16)
            nc.sync.dma_start(w2[:], w2_d[:])
            w2wt = consts.tile([128, HC, H], bf16)
            nc.sync.dma_start(w2wt[:], w2wt_d[:])
            w1ft = consts.tile([128, HC, S], bf16)
            nc.sync.dma_start(w1ft[:], w1ft_d[:])
            gm = consts.tile([17, S], bf16)
            nc.sync.dma_start(gm[:], gm_d[:])
            b2v = consts.tile([128, HC], f32)
            nc.sync.dma_start(b2v[:], b2v_d[:])

            # z2 (bf16) for the whole core batch, persisted across the
            # activation-table switch: [partition, hidden-chunk, slice, col]
            z2s = consts.tile([128, HC, NS, NSLICE], bf16)

            def load_x_rt(n, tag):
                """x slice replicated at partition offsets 0/32/64/96 for
                row-tiled z1 matmuls."""
                csl = slice(n * NSLICE, (n + 1) * NSLICE)
                x_t = work.tile(
                    [128, NSLICE], bf16, tag="xa", bufs=3, name=f"x{tag}"
                )
                for j in range(4):
                    nc.sync.dma_start(x_t[32 * j : 32 * j + 7, :], xT_d[:, csl])
                return x_t

            def z1_matmuls(x_t, zp):
                """4 K=7 matmuls packed into 4 concurrent PE row groups,
                each writing its own PSUM bank of zp [128, HC*NSLICE]."""
                for j in range(4):
                    nc.tensor.matmul(
                        zp[:, j * NSLICE : (j + 1) * NSLICE],
                        w1rt[32 * j : 32 * j + 7, :],
                        x_t[32 * j : 32 * j + 7, :],
                        start=True,
                        stop=True,
                        tile_position=(32 * j, 0),
                    )

            # ================= stage A: forward to z2 =================
            for n in range(NS):
                x_t = load_x_rt(n, f"a{n}")
                z1p = ps.tile(
                    [128, HC * NSLICE], f32, tag="psa", bufs=1, name=f"z1p{n}"
                )
                z1_matmuls(x_t, z1p)

                # e1 = Exp(z1); h1 = Ln(e1 + 1) = softplus(z1)
                e1 = work.tile(
                    [128, HC, NSLICE], bf16, tag="e1", bufs=1, name=f"e1_{n}"
                )
                nc.scalar.activation(e1[:], z1p[:], EXP)
                h1 = work.tile(
                    [128, HC, NSLICE], bf16, tag="h1", bufs=3, name=f"h1_{n}"
                )
                nc.scalar.activation(h1[:], e1[:], LN, bias=1.0)

                # z2 chunk ic = sum_kc W2[kc,ic].T @ h1[kc]; store (+b2) bf16
                for ic in range(HC):
                    z2p = ps.tile(
                        [128, NSLICE], f32, tag="psb", bufs=4, name=f"z2p{n}_{ic}"
                    )
                    for kc in range(HC):
                        nc.tensor.matmul(
                            z2p[:],
                            w2[:, kc, ic * 128 : (ic + 1) * 128],
                            h1[:, kc, :],
                            start=(kc == 0),
                            stop=(kc == HC - 1),
                        )
                    nc.vector.tensor_scalar(
                        z2s[:, ic, n, :], z2p[:], b2v[:, ic : ic + 1], None, ADD
                    )

            # ================= stage B: backward to output =================
            for n in range(NS):
                csl = slice(n * NSLICE, (n + 1) * NSLICE)
                x_t = load_x_rt(n, f"b{n}")
                a_t = work.tile([17, NSLICE], bf16, tag="aa", bufs=3, name=f"aa{n}")
                nc.sync.dma_start(a_t[:], aT_d[:, csl])

                z1q = ps.tile(
                    [128, HC * NSLICE], f32, tag="psa", bufs=1, name=f"z1q{n}"
                )
                z1_matmuls(x_t, z1q)

                sg1 = work.tile(
                    [128, HC, NSLICE], bf16, tag="sg1", bufs=2, name=f"sg1_{n}"
                )
                nc.scalar.activation(sg1[:], z1q[:], SIG)

                sg2 = work.tile(
                    [128, HC, NSLICE], bf16, tag="sg2", bufs=3, name=f"sg2_{n}"
                )
                nc.scalar.activation(sg2[:], z2s[:, :, n, :], SIG)

                # u chunk ic = sum_jc (W2*w3).T[jc,ic].T @ sg2[jc];  g1 = u*sg1
                g1 = work.tile(
                    [128, HC, NSLICE], bf16, tag="g1", bufs=2, name=f"g1_{n}"
                )
                for ic in range(HC):
                    up = ps.tile(
                        [128, NSLICE], f32, tag="psb", bufs=4, name=f"up{n}_{ic}"
                    )
                    for jc in range(HC):
                        nc.tensor.matmul(
                            up[:],
                            w2wt[:, jc, ic * 128 : (ic + 1) * 128],
                            sg2[:, jc, :],
                            start=(jc == 0),
                            stop=(jc == HC - 1),
                        )
                    nc.vector.tensor_mul(g1[:, ic, :], up[:], sg1[:, ic, :])

                # outT slice = (M@W1) @ g1 + GM.T @ a  (accumulated in psum)
                op = ps.tile([S, NSLICE], f32, tag="psb", bufs=4, name=f"op{n}")
                for kc in range(HC):
                    nc.tensor.matmul(
                        op[:],
                        w1ft[:, kc, :],
                        g1[:, kc, :],
                        start=(kc == 0),
                        stop=False,
                        skip_group_check=True,
                    )
                nc.tensor.matmul(
                    op[:], gm[:], a_t[:], start=False, stop=True, skip_group_check=True
                )
                o_t = work.tile([S, NSLICE], f32, tag="osb", bufs=3, name=f"ot{n}")
                nc.scalar.copy(o_t[:], op[:])
                nc.sync.dma_start(outT_d[:, csl], o_t[:])

    nc.compile()
    return nc


def _hi_lo(a32):
    hi = a32.astype(BF16)
    lo = (a32 - hi.astype(np.float32)).astype(BF16)
    return hi, lo


def kernel(
    t,
    state,
    action_emb,
    W1,
    b1,
    W2,
    b2,
    W3,
    b3,
    log_damping,
    Gw,
    Gb,
):
    global last_results
    import os
    from concourse.bass_utils import run_bass_kernel_spmd

    state = np.asarray(state, dtype=np.float32)
    action_emb = np.asarray(action_emb, dtype=np.float32)
    W1 = np.asarray(W1, dtype=np.float32)
    b1 = np.asarray(b1, dtype=np.float32)
    W2 = np.asarray(W2, dtype=np.float32)
    b2 = np.asarray(b2, dtype=np.float32)
    W3 = np.asarray(W3, dtype=np.float32)
    b3 = np.asarray(b3, dtype=np.float32)  # unused: constant shift, no grad
    damping = float(np.exp(np.float32(log_damping)))
    Gw = np.asarray(Gw, dtype=np.float32)
    Gb = np.asarray(Gb, dtype=np.float32)

    # ---- host-side weight prep (tiny) ----
    w3col = W3[:, 0]
    w1hi, w1lo = _hi_lo(W1)  # [2, H] each
    w1a = np.concatenate(
        [w1hi, w1hi, w1lo, b1[None, :].astype(BF16)], axis=0
    )  # [7, H] bf16
    # row-tiled layout: rows 32j+r = w1a[r, 128j:128j+128]
    w1rt = np.zeros((128, 128), dtype=BF16)
    for j in range(4):
        w1rt[32 * j : 32 * j + 7, :] = w1a[:, 128 * j : 128 * (j + 1)]

    w2r = (
        W2.astype(BF16).reshape(HC, 128, H).transpose(1, 0, 2).copy()
    )  # [128, HC, H]; [p, kc, i] = W2[kc*128+p, i]
    w2wt = (W2.T * w3col[:, None]).astype(BF16)  # [H(j), H(i)]
    w2wtr = w2wt.reshape(HC, 128, H).transpose(1, 0, 2).copy()

    M = np.array([[0.0, 1.0], [-1.0, -damping]], dtype=np.float32)
    w1f = M @ W1  # [2, H]
    w1ftr = w1f.T.astype(BF16).reshape(HC, 128, S).transpose(1, 0, 2).copy()

    gmat = np.zeros((17, S), dtype=np.float32)
    gmat[0:8, 1] = Gw[:, 0]
    gmat[8:16, 1] = Gw[:, 0]
    gmat[16, 1] = Gb[0]
    gmat = gmat.astype(BF16)

    b2v = np.ascontiguousarray(b2.reshape(HC, 128).T)  # [128, HC] f32

    # ---- per-core input shards ----
    sT = state.T  # [2, B]
    shi, slo = _hi_lo(sT)
    ones_row = np.ones((1, B), dtype=BF16)
    xT = np.concatenate([shi, slo, shi, ones_row], axis=0)  # [7, B]

    aT32 = action_emb.T  # [8, B]
    ahi, alo = _hi_lo(aT32)
    aT = np.concatenate([ahi, alo, ones_row], axis=0)  # [17, B]

    if "nc" not in _cached:
        _cached["nc"] = _build_nc()
    nc = _cached["nc"]

    in_maps = []
    for c in range(NCORES):
        csl = slice(c * BC, (c + 1) * BC)
        in_maps.append(
            {
                "xT": np.ascontiguousarray(xT[:, csl]),
                "aT": np.ascontiguousarray(aT[:, csl]),
                "w1rt": w1rt,
                "w2": w2r,
                "w2wt": w2wtr,
                "w1ft": w1ftr,
                "gm": gmat,
                "b2v": b2v,
            }
        )

    trace = bool(os.environ.get("PH_TRACE"))
    res = run_bass_kernel_spmd(
        nc, in_maps, core_ids=list(range(NCORES)), trace=trace
    )
    last_results = res

    out = np.empty((B, S), dtype=np.float32)
    for c in range(NCORES):
        out[c * BC : (c + 1) * BC, :] = res.results[c]["outT"].T
    return out


# revision 4
# speedup vs baseline: 1.4866x; 1.1185x over previous
"""Port-Hamiltonian model forward pass (dstate/dt) as a Bass/Tile kernel on
8 TRN2 NeuronCores, pure data-parallel over the batch.

Math (per sample, feature-major / transposed layout on chip):
    z1T = W1.T sT + b1                  [512, n]
    h1T = softplus(z1T) = Ln(Exp(z1T)+1)
    z2T = W2.T h1T + b2
    s2T = sigmoid(z2T)                  (W3 folded into backward weights)
    uT  = (W2 * w3) s2T                 = dH/dh1 transposed
    g1T = uT * sigmoid(z1T)             = dH/dz1 transposed
    outT = (M @ W1) g1T + GM @ [a_hi; a_lo; 1]
  where M = [[0, 1], [-1, -damping]], GM carries Gw (for G_u) and Gb.

All matmuls in bf16 (1 cyc/row on PE; fp32 is 4). z1 accuracy is recovered
with a hi/lo bf16 split of state and W1 packed into the K dim (K=7:
W1hi*xhi + W1hi*xlo + W1lo*xhi + b1*1), and the 4 hidden-chunk z1 matmuls
run CONCURRENTLY in the PE array via row tiling (tile_position=(32j,0),
each writing a different PSUM bank).  z2T is stored in SBUF (bf16) across
the activation-table switch (exp/ln set -> sigmoid set); z1 is recomputed
in stage B instead of stored (4 packed matmuls/slice beat the DVE copy +
64KB/partition of SBUF).
"""

import numpy as np
import ml_dtypes

B = 131072
S = 2
H = 512
E = 8
NCORES = 8
BC = B // NCORES  # 16384 samples per core
NSLICE = 512      # batch slice (matmul moving free dim / PSUM bank)
NS = BC // NSLICE  # 32 slices
HC = H // 128      # 4 hidden-dim chunks of 128 partitions

BF16 = ml_dtypes.bfloat16

_cached = {}
last_results = None  # test.py introspects this for profiling info


def _pin_act_tables():
    """Restrict the activation-table chooser to the two sets this kernel
    wants (exp+ln together; sigmoid) so Bacc's insert_act_table_loads
    doesn't ping-pong between exp_and_others / natural_log every slice.
    Set ids are positional, so unwanted sets are emptied, not removed."""
    import functools
    import concourse.hw_specs as hw_specs
    import concourse.bacc as bacc

    if getattr(hw_specs.get_activation_tables, "_ph_pinned", False):
        return
    orig = hw_specs.get_activation_tables
    KEEP = {"natural_log_exp_and_others", "sigmoid_and_others"}

    @functools.cache
    def pinned(module_arch):
        full = orig(module_arch)
        return {n: (f if n in KEEP else set()) for n, f in full.items()}

    pinned._ph_pinned = True
    hw_specs.get_activation_tables = pinned
    bacc.get_activation_tables = pinned


def _build_nc():
    import concourse.bacc as bacc
    import concourse.mybir as mybir
    import concourse.tile as tile

    _pin_act_tables()

    f32 = mybir.dt.float32
    bf16 = mybir.dt.bfloat16
    ADD = mybir.AluOpType.add
    EXP = mybir.ActivationFunctionType.Exp
    LN = mybir.ActivationFunctionType.Ln
    SIG = mybir.ActivationFunctionType.Sigmoid

    nc = bacc.Bacc("TRN2", target_bir_lowering=False, debug=False)

    xT_d = nc.dram_tensor("xT", [7, BC], bf16, kind="ExternalInput")
    aT_d = nc.dram_tensor("aT", [17, BC], bf16, kind="ExternalInput")
    # W1-aug row-tiled: rows 32j+r (r<7) hold [W1hi;W1hi;W1lo;b1][r, 128j:128j+128]
    w1rt_d = nc.dram_tensor("w1rt", [128, 128], bf16, kind="ExternalInput")
    w2_d = nc.dram_tensor("w2", [128, HC, H], bf16, kind="ExternalInput")
    w2wt_d = nc.dram_tensor("w2wt", [128, HC, H], bf16, kind="ExternalInput")
    w1ft_d = nc.dram_tensor("w1ft", [128, HC, S], bf16, kind="ExternalInput")
    gm_d = nc.dram_tensor("gm", [17, S], bf16, kind="ExternalInput")
    b2v_d = nc.dram_tensor("b2v", [128, HC], f32, kind="ExternalInput")
    outT_d = nc.dram_tensor("outT", [S, BC], f32, kind="ExternalOutput")

    with tile.TileContext(nc) as tc:
        with (
            tc.tile_pool(name="consts", bufs=1) as consts,
            tc.tile_pool(name="work", bufs=2) as work,
            tc.tile_pool(name="ps", bufs=1, space="PSUM") as ps,
        ):
            # ---- constants ----
            w1rt = consts.tile([128, 128], bf16)
            nc.sync.dma_start(w1rt[:], w1rt_d[:])
            w2 = consts.tile([128, HC, H], bf16)
            nc.sync.dma_start(w2[:], w2_d[:])
            w2wt = consts.tile([128, HC, H], bf16)
            nc.sync.dma_start(w2wt[:], w2wt_d[:])
            w1ft = consts.tile([128, HC, S], bf16)
            nc.sync.dma_start(w1ft[:], w1ft_d[:])
            gm = consts.tile([17, S], bf16)
            nc.sync.dma_start(gm[:], gm_d[:])
            b2v = consts.tile([128, HC], f32)
            nc.sync.dma_start(b2v[:], b2v_d[:])

            # z2 (bf16) for the whole core batch, persisted across the
            # activation-table switch: [partition, hidden-chunk, slice, col]
            z2s = consts.tile([128, HC, NS, NSLICE], bf16)

            def load_x_rt(n, tag):
                """x slice replicated at partition offsets 0/32/64/96 for
                row-tiled z1 matmuls."""
                csl = slice(n * NSLICE, (n + 1) * NSLICE)
                x_t = work.tile(
                    [128, NSLICE], bf16, tag="xa", bufs=3, name=f"x{tag}"
                )
                for j in range(4):
                    nc.sync.dma_start(x_t[32 * j : 32 * j + 7, :], xT_d[:, csl])
                return x_t

            def z1_matmuls(x_t, zp):
                """4 K=7 matmuls packed into 4 concurrent PE row groups,
                each writing its own PSUM bank of zp [128, HC*NSLICE]."""
                for j in range(4):
                    nc.tensor.matmul(
                        zp[:, j * NSLICE : (j + 1) * NSLICE],
                        w1rt[32 * j : 32 * j + 7, :],
                        x_t[32 * j : 32 * j + 7, :],
                        start=True,
                        stop=True,
                        tile_position=(32 * j, 0),
                    )

            # ================= stage A: forward to z2 =================
            for n in range(NS):
                x_t = load_x_rt(n, f"a{n}")
                z1p = ps.tile(
                    [128, HC * NSLICE], f32, tag="psa", bufs=1, name=f"z1p{n}"
                )
                z1_matmuls(x_t, z1p)

                # e1 = Exp(z1); h1 = Ln(e1 + 1) = softplus(z1)
                e1 = work.tile(
                    [128, HC, NSLICE], bf16, tag="e1", bufs=1, name=f"e1_{n}"
                )
                nc.scalar.activation(e1[:], z1p[:], EXP)
                h1 = work.tile(
                    [128, HC, NSLICE], bf16, tag="h1", bufs=3, name=f"h1_{n}"
                )
                nc.scalar.activation(h1[:], e1[:], LN, bias=1.0)

                # z2 chunk ic = sum_kc W2[kc,ic].T @ h1[kc]; store (+b2) bf16
                for ic in range(HC):
                    z2p = ps.tile(
                        [128, NSLICE], f32, tag="psb", bufs=4, name=f"z2p{n}_{ic}"
                    )
                    for kc in range(HC):
                        nc.tensor.matmul(
                            z2p[:],
                            w2[:, kc, ic * 128 : (ic + 1) * 128],
                            h1[:, kc, :],
                            start=(kc == 0),
                            stop=(kc == HC - 1),
                        )
                    nc.vector.tensor_scalar(
                        z2s[:, ic, n, :], z2p[:], b2v[:, ic : ic + 1], None, ADD
                    )

            # ================= stage B: backward to output =================
            for n in range(NS):
                csl = slice(n * NSLICE, (n + 1) * NSLICE)
                x_t = load_x_rt(n, f"b{n}")
                a_t = work.tile([17, NSLICE], bf16, tag="aa", bufs=3, name=f"aa{n}")
                nc.sync.dma_start(a_t[:], aT_d[:, csl])

                z1q = ps.tile(
                    [128, HC * NSLICE], f32, tag="psa", bufs=1, name=f"z1q{n}"
                )
                z1_matmuls(x_t, z1q)

                sg1 = work.tile(
                    [128, HC, NSLICE], bf16, tag="sg1", bufs=2, name=f"sg1_{n}"
                )
                nc.scalar.activation(sg1[:], z1q[:], SIG)

                sg2 = work.tile(
                    [128, HC, NSLICE], bf16, tag="sg2", bufs=3, name=f"sg2_{n}"
                )
                nc.scalar.activation(sg2[:], z2s[:, :, n, :], SIG)

                # u chunk ic = sum_jc (W2*w3).T[jc,ic].T @ sg2[jc];  g1 = u*sg1
                g1 = work.tile(
                    [128, HC, NSLICE], bf16, tag="g1", bufs=2, name=f"g1_{n}"
                )
                for ic in range(HC):
                    up = ps.tile(
                        [128, NSLICE], f32, tag="psb", bufs=4, name=f"up{n}_{ic}"
                    )
                    for jc in range(HC):
                        nc.tensor.matmul(
                            up[:],
                            w2wt[:, jc, ic * 128 : (ic + 1) * 128],
                            sg2[:, jc, :],
                            start=(jc == 0),
                            stop=(jc == HC - 1),
                        )
                    nc.vector.tensor_mul(g1[:, ic, :], up[:], sg1[:, ic, :])

                # outT slice = (M@W1) @ g1 + GM.T @ a  (accumulated in psum)
                op = ps.tile([S, NSLICE], f32, tag="psb", bufs=4, name=f"op{n}")
                for kc in range(HC):
                    nc.tensor.matmul(
                        op[:],
                        w1ft[:, kc, :],
                        g1[:, kc, :],
                        start=(kc == 0),
                        stop=False,
                        skip_group_check=True,
                    )
                nc.tensor.matmul(
                    op[:], gm[:], a_t[:], start=False, stop=True, skip_group_check=True
                )
                o_t = work.tile([S, NSLICE], f32, tag="osb", bufs=3, name=f"ot{n}")
                nc.scalar.copy(o_t[:], op[:])
                nc.sync.dma_start(outT_d[:, csl], o_t[:])

    nc.compile()
    return nc


def _hi_lo(a32):
    hi = a32.astype(BF16)
    lo = (a32 - hi.astype(np.float32)).astype(BF16)
    return hi, lo


def kernel(
    t,
    state,
    action_emb,
    W1,
    b1,
    W2,
    b2,
    W3,
    b3,
    log_damping,
    Gw,
    Gb,
):
    global last_results
    import os
    from concourse.bass_utils import run_bass_kernel_spmd

    state = np.asarray(state, dtype=np.float32)
    action_emb = np.asarray(action_emb, dtype=np.float32)
    W1 = np.asarray(W1, dtype=np.float32)
    b1 = np.asarray(b1, dtype=np.float32)
    W2 = np.asarray(W2, dtype=np.float32)
    b2 = np.asarray(b2, dtype=np.float32)
    W3 = np.asarray(W3, dtype=np.float32)
    b3 = np.asarray(b3, dtype=np.float32)  # unused: constant shift, no grad
    damping = float(np.exp(np.float32(log_damping)))
    Gw = np.asarray(Gw, dtype=np.float32)
    Gb = np.asarray(Gb, dtype=np.float32)

    # ---- host-side weight prep (tiny) ----
    w3col = W3[:, 0]
    w1hi, w1lo = _hi_lo(W1)  # [2, H] each
    w1a = np.concatenate(
        [w1hi, w1hi, w1lo, b1[None, :].astype(BF16)], axis=0
    )  # [7, H] bf16
    # row-tiled layout: rows 32j+r = w1a[r, 128j:128j+128]
    w1rt = np.zeros((128, 128), dtype=BF16)
    for j in range(4):
        w1rt[32 * j : 32 * j + 7, :] = w1a[:, 128 * j : 128 * (j + 1)]

    w2r = (
        W2.astype(BF16).reshape(HC, 128, H).transpose(1, 0, 2).copy()
    )  # [128, HC, H]; [p, kc, i] = W2[kc*128+p, i]
    w2wt = (W2.T * w3col[:, None]).astype(BF16)  # [H(j), H(i)]
    w2wtr = w2wt.reshape(HC, 128, H).transpose(1, 0, 2).copy()

    M = np.array([[0.0, 1.0], [-1.0, -damping]], dtype=np.float32)
    w1f = M @ W1  # [2, H]
    w1ftr = w1f.T.astype(BF16).reshape(HC, 128, S).transpose(1, 0, 2).copy()

    gmat = np.zeros((17, S), dtype=np.float32)
    gmat[0:8, 1] = Gw[:, 0]
    gmat[8:16, 1] = Gw[:, 0]
    gmat[16, 1] = Gb[0]
    gmat = gmat.astype(BF16)

    b2v = np.ascontiguousarray(b2.reshape(HC, 128).T)  # [128, HC] f32

    # ---- per-core input shards ----
    sT = state.T  # [2, B]
    shi, slo = _hi_lo(sT)
    ones_row = np.ones((1, B), dtype=BF16)
    xT = np.concatenate([shi, slo, shi, ones_row], axis=0)  # [7, B]

    aT32 = action_emb.T  # [8, B]
    ahi, alo = _hi_lo(aT32)
    aT = np.concatenate([ahi, alo, ones_row], axis=0)  # [17, B]

    if "nc" not in _cached:
        _cached["nc"] = _build_nc()
    nc = _cached["nc"]

    in_maps = []
    for c in range(NCORES):
        csl = slice(c * BC, (c + 1) * BC)
        in_maps.append(
            {
                "xT": np.ascontiguousarray(xT[:, csl]),
                "aT": np.ascontiguousarray(aT[:, csl]),
                "w1rt": w1rt,
                "w2": w2r,
                "w2wt": w2wtr,
                "w1ft": w1ftr,
                "gm": gmat,
                "b2v": b2v,
            }
        )

    trace = bool(os.environ.get("PH_TRACE"))
    res = run_bass_kernel_spmd(
        nc, in_maps, core_ids=list(range(NCORES)), trace=trace
    )
    last_results = res

    out = np.empty((B, S), dtype=np.float32)
    for c in range(NCORES):
        out[c * BC : (c + 1) * BC, :] = res.results[c]["outT"].T
    return out


# revision 7
# speedup vs baseline: 1.6375x; 1.1015x over previous
"""Port-Hamiltonian model forward pass (dstate/dt) as a Bass/Tile kernel on
8 TRN2 NeuronCores, pure data-parallel over the batch.

Math (per sample, feature-major / transposed layout on chip):
    z1T = W1.T sT + b1                  [512, n]
    h1T = softplus(z1T) = Ln(Exp(z1T)+1)
    z2T = W2.T h1T + b2
    s2T = sigmoid(z2T)                  (W3 folded into backward weights)
    uT  = (W2 * w3) s2T                 = dH/dh1 transposed
    g1T = uT * sigmoid(z1T)             = dH/dz1 transposed
    outT = (M @ W1) g1T + GM @ [a_hi; a_lo; 1]
  where M = [[0, 1], [-1, -damping]], GM carries Gw (for G_u) and Gb.

All matmuls in bf16 (1 cyc/row on PE; fp32 is 4). z1 accuracy is recovered
with a hi/lo bf16 split of state and W1 packed into the K dim (K=7:
W1hi*xhi + W1hi*xlo + W1lo*xhi + b1*1), and the 4 hidden-chunk z1 matmuls
run CONCURRENTLY in the PE array via row tiling (tile_position=(32j,0),
each writing a different PSUM bank).  z2T is stored in SBUF (bf16) across
the activation-table switch (exp/ln set -> sigmoid set); z1 is recomputed
in stage B instead of stored (4 packed matmuls/slice beat the DVE copy +
64KB/partition of SBUF).
"""

import numpy as np
import ml_dtypes

B = 131072
S = 2
H = 512
E = 8
NCORES = 8
BC = B // NCORES  # 16384 samples per core
NSLICE = 512      # batch slice (matmul moving free dim / PSUM bank)
NS = BC // NSLICE  # 32 slices
HC = H // 128      # 4 hidden-dim chunks of 128 partitions

BF16 = ml_dtypes.bfloat16

_cached = {}
last_results = None  # test.py introspects this for profiling info


def _pin_act_tables():
    """Restrict the activation-table chooser to the two sets this kernel
    wants (exp+ln together; sigmoid) so Bacc's insert_act_table_loads
    doesn't ping-pong between exp_and_others / natural_log every slice.
    Set ids are positional, so unwanted sets are emptied, not removed."""
    import functools
    import concourse.hw_specs as hw_specs
    import concourse.bacc as bacc

    if getattr(hw_specs.get_activation_tables, "_ph_pinned", False):
        return
    orig = hw_specs.get_activation_tables
    KEEP = {"natural_log_exp_and_others", "sigmoid_and_others"}

    @functools.cache
    def pinned(module_arch):
        full = orig(module_arch)
        return {n: (f if n in KEEP else set()) for n, f in full.items()}

    pinned._ph_pinned = True
    hw_specs.get_activation_tables = pinned
    bacc.get_activation_tables = pinned


def _build_nc():
    import concourse.bacc as bacc
    import concourse.mybir as mybir
    import concourse.tile as tile

    _pin_act_tables()

    f32 = mybir.dt.float32
    bf16 = mybir.dt.bfloat16
    ADD = mybir.AluOpType.add
    EXP = mybir.ActivationFunctionType.Exp
    LN = mybir.ActivationFunctionType.Ln
    SIG = mybir.ActivationFunctionType.Sigmoid

    nc = bacc.Bacc("TRN2", target_bir_lowering=False, debug=False)

    xT_d = nc.dram_tensor("xT", [7, BC], bf16, kind="ExternalInput")
    aT_d = nc.dram_tensor("aT", [17, BC], bf16, kind="ExternalInput")
    # W1-aug row-tiled: rows 32j+r (r<7) hold [W1hi;W1hi;W1lo;b1][r, 128j:128j+128]
    w1rt_d = nc.dram_tensor("w1rt", [128, 128], bf16, kind="ExternalInput")
    w2_d = nc.dram_tensor("w2", [128, HC, H], bf16, kind="ExternalInput")
    w2wt_d = nc.dram_tensor("w2wt", [128, HC, H], bf16, kind="ExternalInput")
    w1ft_d = nc.dram_tensor("w1ft", [128, HC, S], bf16, kind="ExternalInput")
    gm_d = nc.dram_tensor("gm", [17, S], bf16, kind="ExternalInput")
    b2v_d = nc.dram_tensor("b2v", [128, HC], f32, kind="ExternalInput")
    outT_d = nc.dram_tensor("outT", [S, BC], f32, kind="ExternalOutput")

    with tile.TileContext(nc) as tc:
        with (
            tc.tile_pool(name="consts", bufs=1) as consts,
            tc.tile_pool(name="work", bufs=2) as work,
            tc.tile_pool(name="ps", bufs=1, space="PSUM") as ps,
        ):
            # ---- constants ----
            w1rt = consts.tile([128, 128], bf16)
            nc.sync.dma_start(w1rt[:], w1rt_d[:])
            w2 = consts.tile([128, HC, H], bf16)
            nc.sync.dma_start(w2[:], w2_d[:])
            w2wt = consts.tile([128, HC, H], bf16)
            nc.sync.dma_start(w2wt[:], w2wt_d[:])
            w1ft = consts.tile([128, HC, S], bf16)
            nc.sync.dma_start(w1ft[:], w1ft_d[:])
            gm = consts.tile([17, S], bf16)
            nc.sync.dma_start(gm[:], gm_d[:])
            b2v = consts.tile([128, HC], f32)
            nc.sync.dma_start(b2v[:], b2v_d[:])

            # z2 (bf16) for the whole core batch, persisted across the
            # activation-table switch: [partition, hidden-chunk, slice, col]
            z2s = consts.tile([128, HC, NS, NSLICE], bf16)

            LG = 2  # slices per input-DMA group
            LX = LG * NSLICE

            def load_x_rt(g, tag):
                """x slice-group replicated at partition offsets 0/32/64/96
                for row-tiled z1 matmuls."""
                csl = slice(g * LX, (g + 1) * LX)
                x_t = work.tile([128, LX], bf16, tag="xa", bufs=2, name=f"x{tag}")
                for j in range(4):
                    nc.sync.dma_start(x_t[32 * j : 32 * j + 7, :], xT_d[:, csl])
                return x_t

            def z1_matmuls(x_t, s, zpa, zpb, tag):
                """4 K=7 matmuls packed into 4 concurrent PE row groups;
                chunk j lands in bank j%2 of half-tile zpa/zpb [128, 2*NSLICE].
                s = slice index within the x_t load group."""
                for j in range(4):
                    zp = zpa if j < 2 else zpb
                    nc.tensor.matmul(
                        zp[:, (j % 2) * NSLICE : (j % 2 + 1) * NSLICE],
                        w1rt[32 * j : 32 * j + 7, :],
                        x_t[32 * j : 32 * j + 7, s * NSLICE : (s + 1) * NSLICE],
                        start=True,
                        stop=True,
                        tile_position=(32 * j, 0),
                    )

            # ================= stage A: forward to z2 =================
            for n in range(NS):
                if n % LG == 0:
                    x_t = load_x_rt(n // LG, f"a{n}")
                z1pa = ps.tile(
                    [128, 2 * NSLICE], f32, tag="psa", bufs=2, name=f"z1pa{n}"
                )
                z1pb = ps.tile(
                    [128, 2 * NSLICE], f32, tag="psa", bufs=2, name=f"z1pb{n}"
                )
                z1_matmuls(x_t, n % LG, z1pa, z1pb, f"a{n}")

                # e1 = Exp(z1); h1 = Ln(e1 + 1) = softplus(z1)
                e1 = work.tile(
                    [128, HC, NSLICE], bf16, tag="e1", bufs=1, name=f"e1_{n}"
                )
                nc.scalar.activation(e1[:, 0:2, :], z1pa[:], EXP)
                nc.scalar.activation(e1[:, 2:4, :], z1pb[:], EXP)
                h1 = work.tile(
                    [128, HC, NSLICE], bf16, tag="h1", bufs=3, name=f"h1_{n}"
                )
                nc.scalar.activation(h1[:], e1[:], LN, bias=1.0)

                # z2 chunk ic = sum_kc W2[kc,ic].T @ h1[kc]; store (+b2) bf16
                for ic in range(HC):
                    z2p = ps.tile(
                        [128, NSLICE], f32, tag="psb", bufs=4, name=f"z2p{n}_{ic}"
                    )
                    for kc in range(HC):
                        nc.tensor.matmul(
                            z2p[:],
                            w2[:, kc, ic * 128 : (ic + 1) * 128],
                            h1[:, kc, :],
                            start=(kc == 0),
                            stop=(kc == HC - 1),
                        )
                    nc.vector.tensor_scalar(
                        z2s[:, ic, n, :], z2p[:], b2v[:, ic : ic + 1], None, ADD
                    )

            # ================= stage B: backward to output =================
            for n in range(NS):
                csl = slice(n * NSLICE, (n + 1) * NSLICE)
                if n % LG == 0:
                    x_t = load_x_rt(n // LG, f"b{n}")
                    a_t = work.tile([17, LX], bf16, tag="aa", bufs=2, name=f"aa{n}")
                    nc.sync.dma_start(a_t[:], aT_d[:, n * NSLICE : n * NSLICE + LX])

                z1qa = ps.tile(
                    [128, 2 * NSLICE], f32, tag="psa", bufs=2, name=f"z1qa{n}"
                )
                z1qb = ps.tile(
                    [128, 2 * NSLICE], f32, tag="psa", bufs=2, name=f"z1qb{n}"
                )
                z1_matmuls(x_t, n % LG, z1qa, z1qb, f"b{n}")

                sg1 = work.tile(
                    [128, HC, NSLICE], bf16, tag="sg1", bufs=2, name=f"sg1_{n}"
                )
                nc.scalar.activation(sg1[:, 0:2, :], z1qa[:], SIG)
                nc.scalar.activation(sg1[:, 2:4, :], z1qb[:], SIG)

                sg2 = work.tile(
                    [128, HC, NSLICE], bf16, tag="sg2", bufs=3, name=f"sg2_{n}"
                )
                nc.scalar.activation(sg2[:], z2s[:, :, n, :], SIG)

                # u chunk ic = sum_jc (W2*w3).T[jc,ic].T @ sg2[jc];  g1 = u*sg1
                g1 = work.tile(
                    [128, HC, NSLICE], bf16, tag="g1", bufs=2, name=f"g1_{n}"
                )
                for ic in range(HC):
                    up = ps.tile(
                        [128, NSLICE], f32, tag="psb", bufs=4, name=f"up{n}_{ic}"
                    )
                    for jc in range(HC):
                        nc.tensor.matmul(
                            up[:],
                            w2wt[:, jc, ic * 128 : (ic + 1) * 128],
                            sg2[:, jc, :],
                            start=(jc == 0),
                            stop=(jc == HC - 1),
                        )
                    nc.vector.tensor_mul(g1[:, ic, :], up[:], sg1[:, ic, :])

                # outT slice = (M@W1) @ g1 + GM.T @ a  (accumulated in psum)
                op = ps.tile([S, NSLICE], f32, tag="psb", bufs=4, name=f"op{n}")
                for kc in range(HC):
                    nc.tensor.matmul(
                        op[:],
                        w1ft[:, kc, :],
                        g1[:, kc, :],
                        start=(kc == 0),
                        stop=False,
                        skip_group_check=True,
                    )
                nc.tensor.matmul(
                    op[:],
                    gm[:],
                    a_t[:, (n % LG) * NSLICE : (n % LG + 1) * NSLICE],
                    start=False,
                    stop=True,
                    skip_group_check=True,
                )
                o_t = work.tile([S, NSLICE], f32, tag="osb", bufs=3, name=f"ot{n}")
                nc.scalar.copy(o_t[:], op[:])
                nc.sync.dma_start(outT_d[:, csl], o_t[:])

    nc.compile()
    return nc


def _hi_lo(a32):
    hi = a32.astype(BF16)
    lo = (a32 - hi.astype(np.float32)).astype(BF16)
    return hi, lo


def kernel(
    t,
    state,
    action_emb,
    W1,
    b1,
    W2,
    b2,
    W3,
    b3,
    log_damping,
    Gw,
    Gb,
):
    global last_results
    import os
    from concourse.bass_utils import run_bass_kernel_spmd

    state = np.asarray(state, dtype=np.float32)
    action_emb = np.asarray(action_emb, dtype=np.float32)
    W1 = np.asarray(W1, dtype=np.float32)
    b1 = np.asarray(b1, dtype=np.float32)
    W2 = np.asarray(W2, dtype=np.float32)
    b2 = np.asarray(b2, dtype=np.float32)
    W3 = np.asarray(W3, dtype=np.float32)
    b3 = np.asarray(b3, dtype=np.float32)  # unused: constant shift, no grad
    damping = float(np.exp(np.float32(log_damping)))
    Gw = np.asarray(Gw, dtype=np.float32)
    Gb = np.asarray(Gb, dtype=np.float32)

    # ---- host-side weight prep (tiny) ----
    w3col = W3[:, 0]
    w1hi, w1lo = _hi_lo(W1)  # [2, H] each
    w1a = np.concatenate(
        [w1hi, w1hi, w1lo, b1[None, :].astype(BF16)], axis=0
    )  # [7, H] bf16
    # row-tiled layout: rows 32j+r = w1a[r, 128j:128j+128]
    w1rt = np.zeros((128, 128), dtype=BF16)
    for j in range(4):
        w1rt[32 * j : 32 * j + 7, :] = w1a[:, 128 * j : 128 * (j + 1)]

    w2r = (
        W2.astype(BF16).reshape(HC, 128, H).transpose(1, 0, 2).copy()
    )  # [128, HC, H]; [p, kc, i] = W2[kc*128+p, i]
    w2wt = (W2.T * w3col[:, None]).astype(BF16)  # [H(j), H(i)]
    w2wtr = w2wt.reshape(HC, 128, H).transpose(1, 0, 2).copy()

    M = np.array([[0.0, 1.0], [-1.0, -damping]], dtype=np.float32)
    w1f = M @ W1  # [2, H]
    w1ftr = w1f.T.astype(BF16).reshape(HC, 128, S).transpose(1, 0, 2).copy()

    gmat = np.zeros((17, S), dtype=np.float32)
    gmat[0:8, 1] = Gw[:, 0]
    gmat[8:16, 1] = Gw[:, 0]
    gmat[16, 1] = Gb[0]
    gmat = gmat.astype(BF16)

    b2v = np.ascontiguousarray(b2.reshape(HC, 128).T)  # [128, HC] f32

    # ---- per-core input shards ----
    sT = state.T  # [2, B]
    shi, slo = _hi_lo(sT)
    ones_row = np.ones((1, B), dtype=BF16)
    xT = np.concatenate([shi, slo, shi, ones_row], axis=0)  # [7, B]

    aT32 = action_emb.T  # [8, B]
    ahi, alo = _hi_lo(aT32)
    aT = np.concatenate([ahi, alo, ones_row], axis=0)  # [17, B]

    if "nc" not in _cached:
        _cached["nc"] = _build_nc()
    nc = _cached["nc"]

    in_maps = []
    for c in range(NCORES):
        csl = slice(c * BC, (c + 1) * BC)
        in_maps.append(
            {
                "xT": np.ascontiguousarray(xT[:, csl]),
                "aT": np.ascontiguousarray(aT[:, csl]),
                "w1rt": w1rt,
                "w2": w2r,
                "w2wt": w2wtr,
                "w1ft": w1ftr,
                "gm": gmat,
                "b2v": b2v,
            }
        )

    trace = bool(os.environ.get("PH_TRACE"))
    res = run_bass_kernel_spmd(
        nc, in_maps, core_ids=list(range(NCORES)), trace=trace
    )
    last_results = res

    out = np.empty((B, S), dtype=np.float32)
    for c in range(NCORES):
        out[c * BC : (c + 1) * BC, :] = res.results[c]["outT"].T
    return out
